# Initial kernel scaffold
#
"""TransE-TYPE scoring kernel for Trainium2 (8 NeuronCores, SPMD).

Reference computation (B=8192 triplets, E=1M entities, D=128):
    h        = entity_emb[sample[:, 0]]                      # [B, 128]
    head_idx = sample[:, 1] * 16 + node_type[sample[:, 0]]   # [B]
    Mh       = type_emb[head_idx].reshape(B, 128, 128)
    he       = einsum('bij,bj->bi', Mh, h)
    he       = he / max(|he|, 1e-12)         (elementwise)
    score    = |he[:,0] + he[:,1] - he[:,2] + 1e-6|          # [B]

Only rows 0..2 of each 128x128 type matrix reach the output, so the kernel
gathers just type_emb[head_idx, 0:384] (first 3 rows, contiguous) instead of
the full 64KB matrix. All gathers run on-device via GPSIMD indirect DMA.

Sharding: data-parallel over the batch. Each of the 8 cores processes 1024
triplets; the entity / node_type / type tables are replicated into each
core's HBM (gathers against random indices stay local, no collectives).

Per-core layout: local sample n (0..1023) lives at partition p = n % 128,
column t = n // 128 (8 chunks of 128). Indirect DMA gathers one row per
partition, so each chunk is one gather instruction.
"""

import numpy as np

import concourse.bass as bass
import concourse.mybir as mybir
import concourse.tile as tile
from concourse.bass import IndirectOffsetOnAxis
from concourse.bass_utils import run_bass_kernel_spmd
from concourse.vector_clock import ScopedClock

P = 128          # partitions
NCORES = 8
B = 8192         # total batch
BC = B // NCORES  # 1024 per core
T = BC // P      # 8 chunks of 128 samples
E = 1_000_000
D = 128
TYPE_ROWS = 3200
TYPE_COLS = 16384
SLICE = 3 * D    # 384 floats of each type row that matter

F32 = mybir.dt.float32
I32 = mybir.dt.int32
ALU = mybir.AluOpType

# ---------------------------------------------------------------------------
# The walrus build in this container rejects instructions carrying more than
# one sync-wait. TileContext's tail drain aggregates every outstanding sem
# wait onto a single Drain, which trips that limit; spread the waits over
# NoOp carriers ahead of the drain instead.
_MAX_WAITS_PER_INST = 1


def _patched_drain_and_barrier(self, tick_clock, wait_clock):
    carrier = self.nc.sync.nop(nofuse=True, hint="tile_tail_waits")
    wait_clock.add_sem_waits(carrier.ins, ScopedClock({None: tick_clock.global_clock}))
    si = carrier.ins.sync_info
    waits = list(si.on_wait) if si and si.on_wait else []
    if len(waits) > _MAX_WAITS_PER_INST:
        carrier.ins.sync_info = mybir.SyncInfo(
            on_wait=waits[:_MAX_WAITS_PER_INST], on_update=list(si.on_update)
        )
        for i in range(_MAX_WAITS_PER_INST, len(waits), _MAX_WAITS_PER_INST):
            extra = self.nc.sync.nop(nofuse=True, hint="tile_tail_waits")
            extra.ins.sync_info = mybir.SyncInfo(
                on_wait=waits[i : i + _MAX_WAITS_PER_INST], on_update=[]
            )
    self.nc.sync.drain()
    self.nc.all_engine_barrier()
    assert self.sems is not None
    popped = self.nc._tile_sem_poison_stack.pop()
    assert popped is self._sem_poison
    self.nc.clear_and_free_semaphores(list(self.sems.allocated().values()))
    self.nc.all_engine_barrier()


tile.TileContext._drain_and_barrier = _patched_drain_and_barrier


def emit_body(tc, pool, sample_ap, entity_ap, ntype_ap, type_ap, score_ap, it=0):
    """One full per-core scoring pass (1024 samples)."""
    nc = tc.nc

    # sample [1024,3] -> tile[p, 3t+c] = sample[t*128+p, c]
    samp = pool.tile([P, 3 * T], I32, tag=f"samp{it}")
    nc.sync.dma_start(
        out=samp[:], in_=sample_ap.rearrange("(t p) c -> p (t c)", p=P)
    )
    samp3 = samp[:].rearrange("p (t c) -> p t c", c=3)
    heads = samp3[:, :, 0]  # [128, 8] int32
    rels = samp3[:, :, 1]

    # node_type[heads] : 8 chunk gathers of one int32 per partition
    nt = pool.tile([P, T], I32, tag=f"nt{it}")
    for t in range(T):
        nc.gpsimd.indirect_dma_start(
            out=nt[:, t : t + 1],
            out_offset=None,
            in_=ntype_ap,
            in_offset=IndirectOffsetOnAxis(ap=heads[:, t : t + 1], axis=0),
        )

    # head_idx = rel*16 + nt, per chunk so each type gather launches early
    idx = pool.tile([P, T], I32, tag=f"idx{it}")
    typ = pool.tile([P, T * SLICE], F32, tag=f"typ{it}")
    typ3 = typ[:].rearrange("p (t f) -> p t f", f=SLICE)
    for t in range(T):
        nc.vector.tensor_scalar(
            out=idx[:, t : t + 1],
            in0=rels[:, t : t + 1],
            scalar1=16,
            scalar2=None,
            op0=ALU.mult,
        )
        nc.vector.tensor_tensor(
            out=idx[:, t : t + 1],
            in0=idx[:, t : t + 1],
            in1=nt[:, t : t + 1],
            op=ALU.add,
        )
        nc.gpsimd.indirect_dma_start(
            out=typ3[:, t, :],
            out_offset=None,
            in_=type_ap,
            in_offset=IndirectOffsetOnAxis(ap=idx[:, t : t + 1], axis=0),
        )

    # entity rows (independent of node_type; emitted after so the Pool engine
    # runs the dependency-critical gathers first)
    ent = pool.tile([P, T * D], F32, tag=f"ent{it}")
    ent3 = ent[:].rearrange("p (t d) -> p t d", d=D)
    for t in range(T):
        nc.gpsimd.indirect_dma_start(
            out=ent3[:, t, :],
            out_offset=None,
            in_=entity_ap,
            in_offset=IndirectOffsetOnAxis(ap=heads[:, t : t + 1], axis=0),
        )

    # he_i[p,t] = sum_d typ[p, t, i*128+d] * ent[p, t, d]   for i in 0..2
    typ4 = typ[:].rearrange("p (t i d) -> p t i d", i=3, d=D)
    he = pool.tile([P, 3 * T], F32, tag=f"he{it}")
    he3 = he[:].rearrange("p (i t) -> p i t", i=3)
    for i in range(3):
        prod = pool.tile([P, T * D], F32, tag=f"prod{it}_{i}")
        prod3 = prod[:].rearrange("p (t d) -> p t d", d=D)
        nc.vector.tensor_tensor(out=prod3, in0=typ4[:, :, i, :], in1=ent3, op=ALU.mult)
        nc.vector.tensor_reduce(
            out=he3[:, i, :], in_=prod3, axis=mybir.AxisListType.X, op=ALU.add
        )

    # s = he / max(|he|, 1e-12)  (reciprocal + multiply)
    am = pool.tile([P, 3 * T], F32, tag=f"am{it}")
    nc.vector.tensor_scalar(
        out=am[:], in0=he[:], scalar1=1e-12, scalar2=None, op0=ALU.abs_max
    )
    rec = pool.tile([P, 3 * T], F32, tag=f"rec{it}")
    nc.vector.reciprocal(out=rec[:], in_=am[:])
    s = pool.tile([P, 3 * T], F32, tag=f"s{it}")
    nc.vector.tensor_tensor(out=s[:], in0=he[:], in1=rec[:], op=ALU.mult)
    s3 = s[:].rearrange("p (i t) -> p i t", i=3)

    # score = |s0 + s1 - s2 + 1e-6|
    sc = pool.tile([P, T], F32, tag=f"sc{it}")
    nc.vector.tensor_tensor(out=sc[:], in0=s3[:, 0, :], in1=s3[:, 1, :], op=ALU.add)
    nc.vector.tensor_tensor(out=sc[:], in0=sc[:], in1=s3[:, 2, :], op=ALU.subtract)
    nc.vector.tensor_scalar(
        out=sc[:], in0=sc[:], scalar1=1e-6, scalar2=0.0, op0=ALU.add, op1=ALU.abs_max
    )

    # score[t*128+p] = sc[p, t]
    nc.sync.dma_start(out=score_ap.rearrange("(t p) -> p t", p=P), in_=sc[:])
    return sc


def build_nc(loop_k: int = 1):
    """Build the per-core Bass program. loop_k > 1 repeats the body (for
    wall-clock slope benchmarking); the graded path uses loop_k=1."""
    nc = bass.Bass("TRN2", target_bir_lowering=False, debug=False)
    sample_t = nc.dram_tensor("sample", [BC, 3], I32, kind="ExternalInput")
    ent_t = nc.dram_tensor("entity_emb", [E, D], F32, kind="ExternalInput")
    nt_t = nc.dram_tensor("node_type", [E, 1], I32, kind="ExternalInput")
    typ_t = nc.dram_tensor("type_emb", [TYPE_ROWS, TYPE_COLS], F32, kind="ExternalInput")
    score_t = nc.dram_tensor("score", [BC], F32, kind="ExternalOutput")

    with tile.TileContext(nc) as tc:
        with tc.tile_pool(name="main", bufs=1) as pool:
            for it in range(loop_k):
                emit_body(
                    tc,
                    pool,
                    sample_t.ap(),
                    ent_t.ap(),
                    nt_t.ap(),
                    typ_t.ap(),
                    score_t.ap(),
                    it=0,  # same tags -> same tiles -> iterations serialize
                )
    return nc


_NC_CACHE = {}


def _get_nc(loop_k: int = 1):
    if loop_k not in _NC_CACHE:
        _NC_CACHE[loop_k] = build_nc(loop_k)
    return _NC_CACHE[loop_k]


def make_in_maps(inputs):
    sample = np.ascontiguousarray(np.asarray(inputs["sample"], dtype=np.int32))
    entity = np.ascontiguousarray(np.asarray(inputs["entity_emb"], dtype=np.float32))
    ntype = np.ascontiguousarray(
        np.asarray(inputs["node_type"], dtype=np.int32).reshape(E, 1)
    )
    typemb = np.ascontiguousarray(np.asarray(inputs["type_emb"], dtype=np.float32))
    return [
        {
            "sample": sample[c * BC : (c + 1) * BC],
            "entity_emb": entity,
            "node_type": ntype,
            "type_emb": typemb,
        }
        for c in range(NCORES)
    ]


def kernel(**inputs) -> np.ndarray:
    nc = _get_nc(1)
    in_maps = make_in_maps(inputs)
    res = run_bass_kernel_spmd(nc, in_maps, core_ids=list(range(NCORES)))
    return np.concatenate([res.results[c]["score"] for c in range(NCORES)])


# revision 9
# speedup vs baseline: 1.5753x; 1.5753x over previous
"""TransE-TYPE scoring kernel for Trainium2 (8 NeuronCores, SPMD).

Reference computation (B=8192 triplets, E=1M entities, D=128):
    h        = entity_emb[sample[:, 0]]                      # [B, 128]
    head_idx = sample[:, 1] * 16 + node_type[sample[:, 0]]   # [B]
    Mh       = type_emb[head_idx].reshape(B, 128, 128)
    he       = einsum('bij,bj->bi', Mh, h)
    he       = he / max(|he|, 1e-12)         (elementwise)
    score    = |he[:,0] + he[:,1] - he[:,2] + 1e-6|          # [B]

Only rows 0..2 of each 128x128 type matrix reach the output, so the kernel
gathers just type_emb[head_idx, 0:384] (first 3 rows, contiguous) instead of
the full 64KB matrix. All gathers run on-device via GPSIMD indirect DMA.

Sharding: data-parallel over the batch. Each of the 8 cores processes 1024
triplets; the entity / node_type / type tables are replicated into each
core's HBM (gathers against random indices stay local, no collectives).

Per-core layout: local sample n (0..1023) lives at partition p = n % 128,
column t = n // 128 (8 chunks of 128). Indirect DMA gathers one row per
partition, so each chunk is one gather instruction.
"""

import numpy as np

import concourse.bass as bass
import concourse.mybir as mybir
import concourse.tile as tile
from concourse.bass import IndirectOffsetOnAxis
from concourse.bass_utils import run_bass_kernel_spmd
from concourse.vector_clock import ScopedClock

P = 128          # partitions
NCORES = 8
B = 8192         # total batch
BC = B // NCORES  # 1024 per core
T = BC // P      # 8 chunks of 128 samples
E = 1_000_000
D = 128
TYPE_ROWS = 3200
TYPE_COLS = 16384
SLICE = 3 * D    # 384 floats of each type row that matter

F32 = mybir.dt.float32
I32 = mybir.dt.int32
ALU = mybir.AluOpType

# ---------------------------------------------------------------------------
# The walrus build in this container rejects instructions carrying more than
# one sync-wait. TileContext's tail drain aggregates every outstanding sem
# wait onto a single Drain, which trips that limit; spread the waits over
# NoOp carriers ahead of the drain instead.
_MAX_WAITS_PER_INST = 1


def _patched_drain_and_barrier(self, tick_clock, wait_clock):
    carrier = self.nc.sync.nop(nofuse=True, hint="tile_tail_waits")
    wait_clock.add_sem_waits(carrier.ins, ScopedClock({None: tick_clock.global_clock}))
    si = carrier.ins.sync_info
    waits = list(si.on_wait) if si and si.on_wait else []
    if len(waits) > _MAX_WAITS_PER_INST:
        carrier.ins.sync_info = mybir.SyncInfo(
            on_wait=waits[:_MAX_WAITS_PER_INST], on_update=list(si.on_update)
        )
        for i in range(_MAX_WAITS_PER_INST, len(waits), _MAX_WAITS_PER_INST):
            extra = self.nc.sync.nop(nofuse=True, hint="tile_tail_waits")
            extra.ins.sync_info = mybir.SyncInfo(
                on_wait=waits[i : i + _MAX_WAITS_PER_INST], on_update=[]
            )
    self.nc.sync.drain()
    self.nc.all_engine_barrier()
    assert self.sems is not None
    popped = self.nc._tile_sem_poison_stack.pop()
    assert popped is self._sem_poison
    self.nc.clear_and_free_semaphores(list(self.sems.allocated().values()))
    self.nc.all_engine_barrier()


tile.TileContext._drain_and_barrier = _patched_drain_and_barrier


def _split_excess_waits(nc, limit=_MAX_WAITS_PER_INST):
    """The walrus build only encodes `limit` sync-waits per instruction.
    Hoist extra waits onto NoOp carriers inserted just before the owning
    instruction on the same engine (same AND semantics, engine is in-order)."""
    nop_id = [0]
    for fn in nc.m.functions:
        for bb in fn.blocks:
            new_insts = []
            for ins in bb.instructions:
                si = ins.sync_info
                waits = list(si.on_wait) if si and si.on_wait else []
                if len(waits) > limit:
                    keep = waits[:limit]
                    extra = waits[limit:]
                    for i in range(0, len(extra), limit):
                        nop = mybir.InstNoOp(
                            name=f"I-waitnop-{nop_id[0]}", hint="split_waits"
                        )
                        nop_id[0] += 1
                        nop.engine = ins.engine
                        nop.sync_info = mybir.SyncInfo(
                            on_wait=extra[i : i + limit], on_update=[]
                        )
                        new_insts.append(nop)
                    ins.sync_info = mybir.SyncInfo(
                        on_wait=keep, on_update=list(si.on_update)
                    )
                new_insts.append(ins)
            bb.instructions = new_insts


def emit_body(tc, pool, sample_ap, entity_ap, ntype_ap, type_ap, score_ap, it=0):
    """One full per-core scoring pass (1024 samples).

    entity_ap is the packed [E, 129] table: 128 embedding floats + the
    entity's node_type (int32 bit-pattern) as column 128. One gather per
    chunk fetches both the h vector and node_type[head]."""
    nc = tc.nc
    DP = D + 1  # packed row width

    # sample [1024,3] -> tile[p, 3t+c] = sample[t*128+p, c]
    samp = pool.tile([P, 3 * T], I32, tag=f"samp{it}")
    samp3 = samp[:].rearrange("p (t c) -> p t c", c=3)
    nc.sync.dma_start(
        out=samp3, in_=sample_ap.rearrange("(t p) c -> p t c", p=P)
    )
    rels = samp3[:, :, 1]

    # packed entity rows: ent[:, t, 0:128] = h, ent[:, t, 128] = node_type bits
    ent = pool.tile([P, T * DP], F32, tag=f"ent{it}")
    ent3 = ent[:].rearrange("p (t d) -> p t d", d=DP)
    for t in range(T):
        nc.gpsimd.indirect_dma_start(
            out=ent3[:, t, :],
            out_offset=None,
            in_=entity_ap,
            in_offset=IndirectOffsetOnAxis(ap=samp[:, 3 * t : 3 * t + 1], axis=0),
        )

    # head_idx = rel*16 + node_type; per chunk so each type gather launches
    # as soon as its entity chunk arrives
    idx = pool.tile([P, T], I32, tag=f"idx{it}")
    typ = pool.tile([P, T * SLICE], F32, tag=f"typ{it}")
    typ3 = typ[:].rearrange("p (t f) -> p t f", f=SLICE)
    ent_i32 = ent[:].bitcast(I32)
    for t in range(T):
        nc.vector.tensor_scalar(
            out=idx[:, t : t + 1],
            in0=rels[:, t : t + 1],
            scalar1=16,
            scalar2=None,
            op0=ALU.mult,
        )
        nc.vector.tensor_tensor(
            out=idx[:, t : t + 1],
            in0=idx[:, t : t + 1],
            in1=ent_i32[:, t * DP + D : t * DP + D + 1],
            op=ALU.add,
        )
        nc.gpsimd.indirect_dma_start(
            out=typ3[:, t, :],
            out_offset=None,
            in_=type_ap,
            in_offset=IndirectOffsetOnAxis(ap=idx[:, t : t + 1], axis=0),
        )

    # he_i[p,t] = sum_d typ[p, t, i*128+d] * ent[p, t, d]   for i in 0..2
    typ4 = typ[:].rearrange("p (t i d) -> p t i d", i=3, d=D)
    he = pool.tile([P, 3 * T], F32, tag=f"he{it}")
    he3 = he[:].rearrange("p (i t) -> p i t", i=3)
    for i in range(3):
        prod = pool.tile([P, T * D], F32, tag=f"prod{it}_{i}")
        prod3 = prod[:].rearrange("p (t d) -> p t d", d=D)
        nc.vector.tensor_tensor(
            out=prod3, in0=typ4[:, :, i, :], in1=ent3[:, :, :D], op=ALU.mult
        )
        nc.vector.tensor_reduce(
            out=he3[:, i, :], in_=prod3, axis=mybir.AxisListType.X, op=ALU.add
        )

    # s = he / max(|he|, 1e-12)  (abs -> max -> reciprocal -> multiply)
    am = pool.tile([P, 3 * T], F32, tag=f"am{it}")
    nc.scalar.activation(am[:], he[:], mybir.ActivationFunctionType.Abs)
    nc.vector.tensor_scalar(
        out=am[:], in0=am[:], scalar1=1e-12, scalar2=None, op0=ALU.max
    )
    rec = pool.tile([P, 3 * T], F32, tag=f"rec{it}")
    nc.vector.reciprocal(out=rec[:], in_=am[:])
    s = pool.tile([P, 3 * T], F32, tag=f"s{it}")
    nc.vector.tensor_tensor(out=s[:], in0=he[:], in1=rec[:], op=ALU.mult)
    s3 = s[:].rearrange("p (i t) -> p i t", i=3)

    # score = |s0 + s1 - s2 + 1e-6|
    sc = pool.tile([P, T], F32, tag=f"sc{it}")
    nc.vector.tensor_tensor(out=sc[:], in0=s3[:, 0, :], in1=s3[:, 1, :], op=ALU.add)
    nc.vector.tensor_tensor(out=sc[:], in0=sc[:], in1=s3[:, 2, :], op=ALU.subtract)
    nc.vector.tensor_scalar(
        out=sc[:], in0=sc[:], scalar1=1e-6, scalar2=None, op0=ALU.add
    )
    nc.scalar.activation(sc[:], sc[:], mybir.ActivationFunctionType.Abs)

    # score[t*128+p] = sc[p, t]
    nc.sync.dma_start(out=score_ap.rearrange("(t p) -> p t", p=P), in_=sc[:])
    return sc


def build_nc(loop_k: int = 1):
    """Build the per-core Bass program. loop_k > 1 repeats the body (for
    wall-clock slope benchmarking); the graded path uses loop_k=1."""
    nc = bass.Bass("TRN2", target_bir_lowering=False, debug=False)
    sample_t = nc.dram_tensor("sample", [BC, 3], I32, kind="ExternalInput")
    ent_t = nc.dram_tensor("entity_emb", [E, D + 1], F32, kind="ExternalInput")
    typ_t = nc.dram_tensor("type_emb", [TYPE_ROWS, TYPE_COLS], F32, kind="ExternalInput")
    score_t = nc.dram_tensor("score", [BC], F32, kind="ExternalOutput")

    with tile.TileContext(nc) as tc:
        with tc.tile_pool(name="main", bufs=1) as pool:
            for it in range(loop_k):
                emit_body(
                    tc,
                    pool,
                    sample_t.ap(),
                    ent_t.ap(),
                    None,
                    typ_t.ap(),
                    score_t.ap(),
                    it=0,  # same tags -> same tiles -> iterations serialize
                )
    _split_excess_waits(nc)
    return nc


_NC_CACHE = {}


def _get_nc(loop_k: int = 1):
    if loop_k not in _NC_CACHE:
        _NC_CACHE[loop_k] = build_nc(loop_k)
    return _NC_CACHE[loop_k]


def make_in_maps(inputs):
    sample = np.ascontiguousarray(np.asarray(inputs["sample"], dtype=np.int32))
    entity = np.asarray(inputs["entity_emb"], dtype=np.float32)
    ntype = np.asarray(inputs["node_type"], dtype=np.int32)
    typemb = np.ascontiguousarray(np.asarray(inputs["type_emb"], dtype=np.float32))
    # pack node_type (int32 bit pattern) as column 128 of the entity table:
    # the per-head entity gather then also delivers node_type[head]
    packed = np.empty((E, D + 1), dtype=np.float32)
    packed[:, :D] = entity
    packed[:, D] = ntype.reshape(E).view(np.float32)
    return [
        {
            "sample": sample[c * BC : (c + 1) * BC],
            "entity_emb": packed,
            "type_emb": typemb,
        }
        for c in range(NCORES)
    ]


def kernel(**inputs) -> np.ndarray:
    nc = _get_nc(1)
    in_maps = make_in_maps(inputs)
    res = run_bass_kernel_spmd(nc, in_maps, core_ids=list(range(NCORES)))
    return np.concatenate([res.results[c]["score"] for c in range(NCORES)])


# revision 10
# speedup vs baseline: 1.8189x; 1.1547x over previous
"""TransE-TYPE scoring kernel for Trainium2 (8 NeuronCores, SPMD).

Reference computation (B=8192 triplets, E=1M entities, D=128):
    h        = entity_emb[sample[:, 0]]                      # [B, 128]
    head_idx = sample[:, 1] * 16 + node_type[sample[:, 0]]   # [B]
    Mh       = type_emb[head_idx].reshape(B, 128, 128)
    he       = einsum('bij,bj->bi', Mh, h)
    he       = he / max(|he|, 1e-12)         (elementwise)
    score    = |he[:,0] + he[:,1] - he[:,2] + 1e-6|          # [B]

Only rows 0..2 of each 128x128 type matrix reach the output, so the kernel
gathers just type_emb[head_idx, 0:384] (first 3 rows, contiguous) instead of
the full 64KB matrix. All gathers run on-device via GPSIMD indirect DMA.

Sharding: data-parallel over the batch. Each of the 8 cores processes 1024
triplets; the entity / node_type / type tables are replicated into each
core's HBM (gathers against random indices stay local, no collectives).

Per-core layout: local sample n (0..1023) lives at partition p = n % 128,
column t = n // 128 (8 chunks of 128). Indirect DMA gathers one row per
partition, so each chunk is one gather instruction.
"""

import numpy as np

import concourse.bass as bass
import concourse.mybir as mybir
import concourse.tile as tile
from concourse.bass import IndirectOffsetOnAxis
from concourse.bass_utils import run_bass_kernel_spmd
from concourse.vector_clock import ScopedClock

P = 128          # partitions
NCORES = 8
B = 8192         # total batch
BC = B // NCORES  # 1024 per core
T = BC // P      # 8 chunks of 128 samples
E = 1_000_000
D = 128
TYPE_ROWS = 3200
TYPE_COLS = 16384
SLICE = 3 * D    # 384 floats of each type row that matter

F32 = mybir.dt.float32
I32 = mybir.dt.int32
ALU = mybir.AluOpType

# ---------------------------------------------------------------------------
# The walrus build in this container rejects instructions carrying more than
# one sync-wait. TileContext's tail drain aggregates every outstanding sem
# wait onto a single Drain, which trips that limit; spread the waits over
# NoOp carriers ahead of the drain instead.
_MAX_WAITS_PER_INST = 1


def _patched_drain_and_barrier(self, tick_clock, wait_clock):
    carrier = self.nc.sync.nop(nofuse=True, hint="tile_tail_waits")
    wait_clock.add_sem_waits(carrier.ins, ScopedClock({None: tick_clock.global_clock}))
    si = carrier.ins.sync_info
    waits = list(si.on_wait) if si and si.on_wait else []
    if len(waits) > _MAX_WAITS_PER_INST:
        carrier.ins.sync_info = mybir.SyncInfo(
            on_wait=waits[:_MAX_WAITS_PER_INST], on_update=list(si.on_update)
        )
        for i in range(_MAX_WAITS_PER_INST, len(waits), _MAX_WAITS_PER_INST):
            extra = self.nc.sync.nop(nofuse=True, hint="tile_tail_waits")
            extra.ins.sync_info = mybir.SyncInfo(
                on_wait=waits[i : i + _MAX_WAITS_PER_INST], on_update=[]
            )
    self.nc.sync.drain()
    self.nc.all_engine_barrier()
    assert self.sems is not None
    popped = self.nc._tile_sem_poison_stack.pop()
    assert popped is self._sem_poison
    self.nc.clear_and_free_semaphores(list(self.sems.allocated().values()))
    self.nc.all_engine_barrier()


tile.TileContext._drain_and_barrier = _patched_drain_and_barrier


def _split_excess_waits(nc, limit=_MAX_WAITS_PER_INST):
    """The walrus build only encodes `limit` sync-waits per instruction.
    Hoist extra waits onto NoOp carriers inserted just before the owning
    instruction on the same engine (same AND semantics, engine is in-order)."""
    nop_id = [0]
    for fn in nc.m.functions:
        for bb in fn.blocks:
            new_insts = []
            for ins in bb.instructions:
                si = ins.sync_info
                waits = list(si.on_wait) if si and si.on_wait else []
                if len(waits) > limit:
                    keep = waits[:limit]
                    extra = waits[limit:]
                    for i in range(0, len(extra), limit):
                        nop = mybir.InstNoOp(
                            name=f"I-waitnop-{nop_id[0]}", hint="split_waits"
                        )
                        nop_id[0] += 1
                        nop.engine = ins.engine
                        nop.sync_info = mybir.SyncInfo(
                            on_wait=extra[i : i + limit], on_update=[]
                        )
                        new_insts.append(nop)
                    ins.sync_info = mybir.SyncInfo(
                        on_wait=keep, on_update=list(si.on_update)
                    )
                new_insts.append(ins)
            bb.instructions = new_insts


def emit_body(tc, pool, sample_ap, entity_ap, ntype_ap, type_ap, score_ap, it=0):
    """One full per-core scoring pass (1024 samples).

    entity_ap is the packed [E, 129] table: 128 embedding floats + the
    entity's node_type (int32 bit-pattern) as column 128. One gather per
    chunk fetches both the h vector and node_type[head]."""
    nc = tc.nc
    DP = D + 1  # packed row width

    # sample [1024,3] -> tile[p, 3t+c] = sample[t*128+p, c]
    samp = pool.tile([P, 3 * T], I32, tag=f"samp{it}")
    samp3 = samp[:].rearrange("p (t c) -> p t c", c=3)
    nc.sync.dma_start(
        out=samp3, in_=sample_ap.rearrange("(t p) c -> p t c", p=P)
    )
    rels = samp3[:, :, 1]

    # packed entity rows: ent[:, t, 0:128] = h, ent[:, t, 128] = node_type bits
    ent = pool.tile([P, T * DP], F32, tag=f"ent{it}")
    ent3 = ent[:].rearrange("p (t d) -> p t d", d=DP)
    for t in range(T):
        nc.gpsimd.indirect_dma_start(
            out=ent3[:, t, :],
            out_offset=None,
            in_=entity_ap,
            in_offset=IndirectOffsetOnAxis(ap=samp[:, 3 * t : 3 * t + 1], axis=0),
        )

    # head_idx = rel*16 + node_type; per chunk so each type gather launches
    # as soon as its entity chunk arrives
    idx = pool.tile([P, T], I32, tag=f"idx{it}")
    typ = pool.tile([P, T * SLICE], F32, tag=f"typ{it}")
    typ3 = typ[:].rearrange("p (t f) -> p t f", f=SLICE)
    ent_i32 = ent[:].bitcast(I32)
    for t in range(T):
        nc.vector.tensor_scalar(
            out=idx[:, t : t + 1],
            in0=rels[:, t : t + 1],
            scalar1=16,
            scalar2=None,
            op0=ALU.mult,
        )
        nc.vector.tensor_tensor(
            out=idx[:, t : t + 1],
            in0=idx[:, t : t + 1],
            in1=ent_i32[:, t * DP + D : t * DP + D + 1],
            op=ALU.add,
        )
        nc.gpsimd.indirect_dma_start(
            out=typ3[:, t, :],
            out_offset=None,
            in_=type_ap,
            in_offset=IndirectOffsetOnAxis(ap=idx[:, t : t + 1], axis=0),
        )

    # Compute in two halves over t so the first half's math overlaps the
    # second half's gathers (shorter tail after the last type chunk lands).
    typ4 = typ[:].rearrange("p (t i d) -> p t i d", i=3, d=D)
    he = pool.tile([P, 3 * T], F32, tag=f"he{it}")
    he3 = he[:].rearrange("p (i t) -> p i t", i=3)
    score2 = score_ap.rearrange("(t p) -> p t", p=P)
    H = T // 2
    for h in range(2):
        ts = slice(h * H, (h + 1) * H)
        # he_i[p,t] = sum_d typ[p, t, i*128+d] * ent[p, t, d]   for i in 0..2
        for i in range(3):
            prod = pool.tile([P, H * D], F32, tag=f"prod{it}_{h}_{i}")
            prod3 = prod[:].rearrange("p (t d) -> p t d", d=D)
            nc.vector.tensor_tensor(
                out=prod3, in0=typ4[:, ts, i, :], in1=ent3[:, ts, :D], op=ALU.mult
            )
            nc.vector.tensor_reduce(
                out=he3[:, i, ts], in_=prod3, axis=mybir.AxisListType.X, op=ALU.add
            )
        heh = he3[:, :, ts]  # [P, 3, H] strided view

        # s = he / max(|he|, 1e-12)  (abs -> max -> reciprocal -> multiply)
        am = pool.tile([P, 3 * H], F32, tag=f"am{it}_{h}")
        am3 = am[:].rearrange("p (i t) -> p i t", i=3)
        nc.scalar.activation(am3, heh, mybir.ActivationFunctionType.Abs)
        nc.vector.tensor_scalar(
            out=am[:], in0=am[:], scalar1=1e-12, scalar2=None, op0=ALU.max
        )
        rec = pool.tile([P, 3 * H], F32, tag=f"rec{it}_{h}")
        nc.vector.reciprocal(out=rec[:], in_=am[:])
        s = pool.tile([P, 3 * H], F32, tag=f"s{it}_{h}")
        s3 = s[:].rearrange("p (i t) -> p i t", i=3)
        nc.vector.tensor_tensor(out=s3, in0=heh, in1=rec[:].rearrange(
            "p (i t) -> p i t", i=3), op=ALU.mult)

        # score = |s0 + s1 - s2 + 1e-6|
        sc = pool.tile([P, H], F32, tag=f"sc{it}_{h}")
        nc.vector.tensor_tensor(
            out=sc[:], in0=s3[:, 0, :], in1=s3[:, 1, :], op=ALU.add
        )
        nc.vector.tensor_tensor(out=sc[:], in0=sc[:], in1=s3[:, 2, :], op=ALU.subtract)
        nc.vector.tensor_scalar(
            out=sc[:], in0=sc[:], scalar1=1e-6, scalar2=None, op0=ALU.add
        )
        nc.scalar.activation(sc[:], sc[:], mybir.ActivationFunctionType.Abs)

        # score[t*128+p] = sc[p, t-h*H]
        nc.sync.dma_start(out=score2[:, ts], in_=sc[:])


def build_nc(loop_k: int = 1):
    """Build the per-core Bass program. loop_k > 1 repeats the body (for
    wall-clock slope benchmarking); the graded path uses loop_k=1."""
    nc = bass.Bass("TRN2", target_bir_lowering=False, debug=False)
    sample_t = nc.dram_tensor("sample", [BC, 3], I32, kind="ExternalInput")
    ent_t = nc.dram_tensor("entity_emb", [E, D + 1], F32, kind="ExternalInput")
    typ_t = nc.dram_tensor("type_emb", [TYPE_ROWS, TYPE_COLS], F32, kind="ExternalInput")
    score_t = nc.dram_tensor("score", [BC], F32, kind="ExternalOutput")

    with tile.TileContext(nc) as tc:
        with tc.tile_pool(name="main", bufs=1) as pool:
            for it in range(loop_k):
                emit_body(
                    tc,
                    pool,
                    sample_t.ap(),
                    ent_t.ap(),
                    None,
                    typ_t.ap(),
                    score_t.ap(),
                    it=0,  # same tags -> same tiles -> iterations serialize
                )
    _split_excess_waits(nc)
    return nc


_NC_CACHE = {}


def _get_nc(loop_k: int = 1):
    if loop_k not in _NC_CACHE:
        _NC_CACHE[loop_k] = build_nc(loop_k)
    return _NC_CACHE[loop_k]


def make_in_maps(inputs):
    sample = np.ascontiguousarray(np.asarray(inputs["sample"], dtype=np.int32))
    entity = np.asarray(inputs["entity_emb"], dtype=np.float32)
    ntype = np.asarray(inputs["node_type"], dtype=np.int32)
    typemb = np.ascontiguousarray(np.asarray(inputs["type_emb"], dtype=np.float32))
    # pack node_type (int32 bit pattern) as column 128 of the entity table:
    # the per-head entity gather then also delivers node_type[head]
    packed = np.empty((E, D + 1), dtype=np.float32)
    packed[:, :D] = entity
    packed[:, D] = ntype.reshape(E).view(np.float32)
    return [
        {
            "sample": sample[c * BC : (c + 1) * BC],
            "entity_emb": packed,
            "type_emb": typemb,
        }
        for c in range(NCORES)
    ]


def kernel(**inputs) -> np.ndarray:
    nc = _get_nc(1)
    in_maps = make_in_maps(inputs)
    res = run_bass_kernel_spmd(nc, in_maps, core_ids=list(range(NCORES)))
    return np.concatenate([res.results[c]["score"] for c in range(NCORES)])
